# revision 1
# baseline (speedup 1.0000x reference)
"""Causal self-attention Bass/Tile kernel for Trainium2, 8 NeuronCores SPMD.

Problem: B=4, T=2048, C=1024, H=16 heads, D=64, f32 in/out.
    qkv = x @ w_qkv.T; per-head causal softmax(q k^T / sqrt(D)) @ v;
    out = attn @ w_out.T + b_out.

Sharding (hybrid batch x tensor-parallel): core c handles batch b = c//2 and
head group hg = c%2 (8 of 16 heads). Each core computes a full [T, C] partial
of the output projection restricted to its heads; the host sums the two
partials per batch and adds the bias.

Per-core device algorithm (all matmuls bf16 x bf16 -> f32 PSUM):
  - qT, kT produced in [j, t] layout, v in [t, j] layout, from xT and wqkvT.
  - scores computed TRANSPOSED: scT[l, i] = k_h q_h^T (keys on partitions), so
    softmax needs no on-chip transposes: exp via ScalarE (scale=1/8 folded,
    no max subtraction -- scores are ~N(0,1), exp can't overflow), causal
    handled by computing only l-blocks <= i and a triangular mask on the
    diagonal 128-blocks.
  - PV: out_h^T[d, i] (+ denominator row) = [v_h | 1]^T @ exp(scT), PSUM-
    accumulated over l-blocks. Row 64 is the softmax denominator.
  - normalization: reciprocal of denoms (batched, DVE Newton), partition-
    broadcast via a DRAM bounce, one elementwise multiply per head.
  - output projection from the (already transposed) attnT with K=128 chunks.

Scheduling: scores PSUM is two double-buffered 2-bank tiles (so ScalarE exp
overlaps the next block's score matmuls) and the PV accumulator holds the
other 4 banks. QKV work for head-pair p+1 is threaded into pair p's
attention stream to fill TensorE gaps while ScalarE is the bottleneck.
"""

import os
import sys

if "/opt/trn_rl_repo" not in sys.path:
    sys.path.insert(0, "/opt/trn_rl_repo")

# debug: "qkv" builds only the projection, "attn" skips the output projection
_PHASES = os.environ.get("KERNEL_PHASES", "all")

import numpy as np
import ml_dtypes

import concourse.bass as bass
import concourse.tile as tile
import concourse.mybir as mybir
from concourse import bacc
from concourse.bass_utils import run_bass_kernel_spmd

BF16 = mybir.dt.bfloat16
F32 = mybir.dt.float32
NPBF16 = ml_dtypes.bfloat16
EXPF = mybir.ActivationFunctionType.Exp

P = 128
C = 1024
CC = C // P      # 8 contraction chunks
NH = 8           # heads per core
D = 64
J = NH * D       # 512 (local q/k/v width)
JC = J // P      # 4 j-chunks


def build_program(T=2048):
    LC = T // P          # l/t 128-blocks
    NS = T // 512        # 512-wide i-supers
    SCALE = 0.125        # 1/sqrt(D)

    nc = bacc.Bacc("TRN2", target_bir_lowering=False, debug=False, num_devices=8)

    xT_d = nc.dram_tensor("xT", [CC, P, T], BF16, kind="ExternalInput")
    wqkvT_d = nc.dram_tensor("wqkvT", [CC, P, 3 * J], BF16, kind="ExternalInput")
    woutT_d = nc.dram_tensor("woutT", [JC, P, C], BF16, kind="ExternalInput")
    mask_d = nc.dram_tensor("trimask", [P, P], BF16, kind="ExternalInput")
    y_d = nc.dram_tensor("y", [LC, P, C], F32, kind="ExternalOutput")

    with tile.TileContext(nc) as tc:
        with (
            tc.tile_pool(name="persist", bufs=1) as persist,
            tc.tile_pool(name="io", bufs=1) as io_pool,
            tc.tile_pool(name="bc", bufs=2) as bc_pool,
            tc.tile_pool(name="dn", bufs=2) as dn_pool,
            tc.tile_pool(name="expp", bufs=3) as exp_pool,
            tc.tile_pool(name="outp", bufs=2) as out_pool,
            tc.tile_pool(name="dramp", bufs=1, space="DRAM") as dram_pool,
            tc.tile_pool(name="ps_a", bufs=2, space="PSUM") as ps_a,
            tc.tile_pool(name="ps_b", bufs=1, space="PSUM") as ps_b,
        ):
            # DRAM bounce buffer for partition-broadcasting the softmax
            # reciprocals (SBUF sources cannot have partition-step-0 APs;
            # DRAM sources can). A pool tile so Tile tracks the RAW hazard
            # between the store and the broadcast load.
            rscr_d = dram_pool.tile([NH, T], BF16)
            woutT = persist.tile([P, JC, C], BF16)
            trimask = persist.tile([P, P], BF16)
            qkT = persist.tile([P, 2 * JC, T], BF16)
            v_aug = persist.tile([P, LC, NH, D + 1], BF16)
            attnT = persist.tile([P, JC, T], BF16)
            # head h's denominator in row h; the reciprocal runs once over all
            # 8 rows at partition base 0 (custom-DVE ops are only HW-proven at
            # base 0 -- nonzero bases returned garbage on silicon).
            denoms = persist.tile([NH, T], F32)
            recips = persist.tile([NH, T], F32)
            rscratch = persist.tile([NH, T], F32)
            recips_bf = persist.tile([NH, T], BF16)
            xT = io_pool.tile([P, CC, T], BF16)
            wqkvT = io_pool.tile([P, CC, 3 * J], BF16)

            for jc in range(JC):
                nc.sync.dma_start(woutT[:, jc, :], woutT_d[jc])
            nc.sync.dma_start(trimask[:], mask_d[:])
            nc.gpsimd.memset(v_aug[:, :, :, D], 1.0)
            for cc in range(CC):
                nc.sync.dma_start(xT[:, cc, :], xT_d[cc])
                nc.sync.dma_start(wqkvT[:, cc, :], wqkvT_d[cc])

            # ---------------- QKV projection pieces ----------------
            # One "pair tile" = a 2-bank PSUM tile holding two 512-wide
            # accumulation groups; all rotate through ps_a (bufs=2).
            def emit_qk_pair(jc, k):
                """q/k chunk jc, t-supers 2k and 2k+1 (clipped to NS)."""
                pq = ps_a.tile([P, 2, 512], F32, tag="sc", name=f"qk{jc}_{k}")
                nts = min(2, NS - 2 * k)
                for i in range(nts):
                    ts = 2 * k + i
                    for cc in range(CC):
                        nc.tensor.matmul(
                            pq[:, i, :],
                            wqkvT[:, cc, jc * P : (jc + 1) * P],
                            xT[:, cc, ts * 512 : (ts + 1) * 512],
                            start=(cc == 0),
                            stop=(cc == CC - 1),
                        )
                nc.vector.tensor_copy(
                    qkT[:, jc, 2 * k * 512 : (2 * k + nts) * 512],
                    pq[:, 0:nts, :].rearrange("p a b -> p (a b)"),
                )

            def emit_v_pair(k):
                """v for t-blocks 2k, 2k+1 into v_aug."""
                pq = ps_a.tile([P, 2, 512], F32, tag="sc", name=f"v{k}")
                for i in range(2):
                    lc = 2 * k + i
                    for cc in range(CC):
                        nc.tensor.matmul(
                            pq[:, i, :],
                            xT[:, cc, lc * P : (lc + 1) * P],
                            wqkvT[:, cc, 2 * J : 3 * J],
                            start=(cc == 0),
                            stop=(cc == CC - 1),
                        )
                nc.vector.tensor_copy(
                    v_aug[:, 2 * k : 2 * k + 2, :, 0:D],
                    pq[:].rearrange("p a (h d) -> p a h d", d=D),
                )

            def qk_pair_tiles(pair):
                """Deferred qk work-items for head pair `pair`."""
                out = []
                for jc in (pair, JC + pair):
                    for k in range((NS + 1) // 2):
                        out.append((jc, k))
                return out

            # pair 0's qk first; v pairs and later pairs' qk are threaded into
            # the attention streams below to keep ScalarE fed from the start.
            for jc, k in qk_pair_tiles(0):
                emit_qk_pair(jc, k)

            # insertion plan: head 0 carries the v projection (v pair k must
            # land before PV consumes l-blocks 2k/2k+1); later heads carry the
            # next pair's qk chunks.
            inserts = {hh: [] for hh in range(NH)}
            for k in range(LC // 2):
                inserts[0].append((max(0, 2 * k - 1), ("v", k)))
            for pair in range(1, JC):
                tiles = qk_pair_tiles(pair)
                carriers = (1,) if pair == 1 else (2 * pair - 2, 2 * pair - 1)
                for i, tl in enumerate(tiles):
                    hh = carriers[i % len(carriers)]
                    inserts[hh].append((None, ("qk", tl)))
            for hh in range(NH):
                items = inserts[hh]
                n_auto = len([it for it in items if it[0] is None])
                auto_pos = [
                    (LC * (i + 1)) // max(1, n_auto) - 1 for i in range(n_auto)
                ]
                fixed = [it for it in items if it[0] is not None]
                autos = [it for it in items if it[0] is None]
                inserts[hh] = sorted(
                    fixed + [(auto_pos[i], autos[i][1]) for i in range(len(autos))]
                )

            # ---------------- attention ----------------
            for h in range(NH if _PHASES != "qkv" else 0):
                bp = (h % 2) * 64
                chq = h // 2
                qTh = qkT[bp : bp + 64, chq, :]
                kTh = qkT[bp : bp + 64, JC + chq, :]
                pv = ps_b.tile([P, NS, 512], F32, tag="pv", name=f"pv{h}")

                # deferred qkv work threaded into this head's pipeline
                insert_at = {}
                for lb_at, item in inserts[h]:
                    insert_at.setdefault(lb_at, []).append(item)

                def emit_scores_exp(lb):
                    """PE score matmuls + ACT exp + DVE diag mask for block lb."""
                    l0 = lb * P
                    ex = exp_pool.tile([P, T], BF16, tag="ex", name=f"ex{h}_{lb}")
                    for tstart in range((l0 // 1024) * 1024, T, 1024):
                        sc = ps_a.tile(
                            [P, 2, 512], F32, tag="sc", name=f"sc{h}_{lb}_{tstart}"
                        )
                        scf = sc[:].rearrange("p a b -> p (a b)")
                        lo = max(l0, tstart)
                        hi = min(tstart + 1024, T)
                        c0 = lo
                        while c0 < hi:
                            n = min(512 - (c0 % 512), hi - c0)
                            nc.tensor.matmul(
                                scf[:, c0 - tstart : c0 - tstart + n],
                                kTh[:, l0 : l0 + P],
                                qTh[:, c0 : c0 + n],
                                start=True,
                                stop=True,
                            )
                            c0 += n
                        nc.scalar.activation(
                            ex[:, lo:hi], scf[:, lo - tstart : hi - tstart],
                            EXPF, scale=SCALE,
                        )
                    # diagonal causal mask on GpSimd: keeps the exp->PV chain
                    # off the DVE queue (which carries the big copies)
                    nc.vector.tensor_mul(
                        ex[:, l0 : l0 + P], ex[:, l0 : l0 + P], trimask[:]
                    )
                    return ex

                def emit_pv(lb, ex):
                    l0 = lb * P
                    for S in range(lb // 4, NS):
                        cs = max(S * 512, l0)
                        n = (S + 1) * 512 - cs
                        nc.tensor.matmul(
                            pv[0 : D + 1, S, cs - S * 512 : cs - S * 512 + n],
                            v_aug[:, lb, h, :],
                            ex[:, cs : cs + n],
                            start=(lb == 0),
                            stop=(lb == 4 * S + 3),
                        )

                # software pipeline: scores(lb+1) and independent qk filler are
                # emitted before PV(lb) so the in-order PE stream never waits
                # on exp(lb).
                ex_prev = emit_scores_exp(0)
                for lb in range(LC):
                    if lb + 1 < LC:
                        ex_cur = emit_scores_exp(lb + 1)
                    for kind, arg in insert_at.get(lb, []):
                        if kind == "v":
                            emit_v_pair(arg)
                        else:
                            emit_qk_pair(*arg)
                    emit_pv(lb, ex_prev)
                    if lb + 1 < LC:
                        ex_prev = ex_cur

                # per-head epilogue: one PSUM read frees the accumulator fast;
                # attnT (bf16) is then cast out of the f32 stage off-path.
                dstage = dn_pool.tile([D + 1, T], F32, tag="dn", name=f"dn{h}")
                nc.vector.tensor_copy(
                    dstage[:], pv[0 : D + 1].rearrange("p a b -> p (a b)")
                )
                nc.vector.tensor_copy(attnT[bp : bp + 64, chq, :], dstage[0:D, :])
                nc.sync.dma_start(denoms[h : h + 1, :], dstage[D : D + 1, :])

            # ---------------- softmax normalization ----------------
            if _PHASES != "qkv":
                nc.vector.reciprocal_approx_accurate(
                    recips[:], denoms[:], rscratch[:]
                )
                nc.vector.tensor_copy(recips_bf[:], recips[:])
                for hh in range(NH):
                    bph = (hh % 2) * 64
                    # both bounce hops on the single SWDGE queue: its FIFO
                    # guarantees the store->broadcast-load order on HW.
                    nc.gpsimd.dma_start(rscr_d[hh], recips_bf[hh : hh + 1, :])
                    bct = bc_pool.tile([P, T], BF16, tag="bc", name=f"bc{hh}")
                    nc.gpsimd.dma_start(
                        bct[bph : bph + 64, :],
                        rscr_d[hh : hh + 1, :].broadcast_to((64, T)),
                    )
                    nc.vector.tensor_mul(
                        attnT[bph : bph + 64, hh // 2, :],
                        attnT[bph : bph + 64, hh // 2, :],
                        bct[bph : bph + 64, :],
                    )

            # ---------------- output projection ----------------
            for tb in range(LC if _PHASES == "all" else 0):
                po = ps_a.tile([P, 2, 512], F32, tag="sc", name=f"o_ps{tb}")
                for oc in range(2):
                    for jc in range(JC):
                        nc.tensor.matmul(
                            po[:, oc, :],
                            attnT[:, jc, tb * P : (tb + 1) * P],
                            woutT[:, jc, oc * 512 : (oc + 1) * 512],
                            start=(jc == 0),
                            stop=(jc == JC - 1),
                        )
                ot = out_pool.tile([P, C], F32, tag="ot", name=f"ot{tb}")
                nc.vector.tensor_copy(
                    ot[:], po[:].rearrange("p a b -> p (a b)")
                )
                nc.sync.dma_start(y_d[tb], ot[:])

    nc.compile()
    return nc


_CACHE = {}

# Set by test harnesses to capture a profile; harmless defaults for grading.
TRACE = False
LAST_RESULT = None


def get_program(T=2048):
    if T not in _CACHE:
        _CACHE[T] = build_program(T)
    return _CACHE[T]


def make_in_map(x_b, w_qkv, w_out, hg, T=2048):
    """Host-side shard prep for one core: batch slice x_b [T, C], head group hg."""
    xT = np.ascontiguousarray(x_b.T).astype(NPBF16).reshape(CC, P, T)
    W = np.concatenate(
        [
            w_qkv[hg * J : (hg + 1) * J],
            w_qkv[C + hg * J : C + (hg + 1) * J],
            w_qkv[2 * C + hg * J : 2 * C + (hg + 1) * J],
        ],
        axis=0,
    )  # [3J, C]
    wqkvT = np.ascontiguousarray(W.T).astype(NPBF16).reshape(CC, P, 3 * J)
    Wo = w_out[:, hg * J : (hg + 1) * J]  # [C, J]
    woutT = np.ascontiguousarray(Wo.T).astype(NPBF16).reshape(JC, P, C)
    tri = np.triu(np.ones((P, P), np.float32)).astype(NPBF16)
    return {"xT": xT, "wqkvT": wqkvT, "woutT": woutT, "trimask": tri}


def kernel(x, w_qkv, w_out, b_out):
    x = np.asarray(x, dtype=np.float32)
    w_qkv = np.asarray(w_qkv, dtype=np.float32)
    w_out = np.asarray(w_out, dtype=np.float32)
    b_out = np.asarray(b_out, dtype=np.float32)
    B, T, Cx = x.shape
    assert Cx == C

    nc = get_program(T)
    in_maps = [
        make_in_map(x[core // 2], w_qkv, w_out, core % 2, T) for core in range(8)
    ]
    res = run_bass_kernel_spmd(nc, in_maps, core_ids=list(range(8)), trace=TRACE)
    global LAST_RESULT
    LAST_RESULT = res
    outs = [r["y"].reshape(T, C).astype(np.float32) for r in res.results]
    y = np.stack([outs[2 * b] + outs[2 * b + 1] for b in range(B)])
    return (y + b_out[None, None, :]).astype(np.float32)



# revision 8
# speedup vs baseline: 1.0831x; 1.0831x over previous
"""Causal self-attention Bass/Tile kernel for Trainium2, 8 NeuronCores SPMD.

Problem: B=4, T=2048, C=1024, H=16 heads, D=64, f32 in/out.
    qkv = x @ w_qkv.T; per-head causal softmax(q k^T / sqrt(D)) @ v;
    out = attn @ w_out.T + b_out.

Sharding (hybrid batch x tensor-parallel): core c handles batch b = c//2 and
head group hg = c%2 (8 of 16 heads). Each core computes a full [T, C] partial
of the output projection restricted to its heads; the host sums the two
partials per batch and adds the bias.

Per-core algorithm, SUPER-MAJOR order (i-supers of 512 query columns):
  for S in 0..3:  (with QKV-projection tiles threaded in as PE filler)
    for h in 0..7:
      for lb in 0..4S+3: scores (kT q, keys on partitions) -> exp -> PV
        - causal diag mask folded into the scores PSUM accumulation as one
          extra 128-col matmul adding -1e6 above the diagonal (ident^T @ M),
          so exp gives exact zeros and no vector-engine hop is needed.
        - PV accumulates [1|v]^T @ ex into a 1-bank [65, 512] PSUM tile;
          row 0 is the softmax denominator (finalized per super by causality).
      normalize (h, S): copy PSUM->SBUF, reciprocal_approx_fast on row 0,
        partition-broadcast the reciprocal row via a K=1 ones matmul into
        PSUM, one fused multiply writes normalized attnT bf16.
    output projection for the 4 t-blocks of S runs during super S+1.

All PSUM tiles are one bank: main pool (scores / qkv pairs / out-proj) x4,
PV accumulators x2, broadcast x2 = 8 banks.
"""

import sys

if "/opt/trn_rl_repo" not in sys.path:
    sys.path.insert(0, "/opt/trn_rl_repo")

import numpy as np
import ml_dtypes

import concourse.bass as bass
import concourse.tile as tile
import concourse.mybir as mybir
from concourse import bacc
from concourse.bass_utils import run_bass_kernel_spmd

BF16 = mybir.dt.bfloat16
F32 = mybir.dt.float32
NPBF16 = ml_dtypes.bfloat16
EXPF = mybir.ActivationFunctionType.Exp

P = 128
C = 1024
CC = C // P      # 8 contraction chunks
NH = 8           # heads per core
D = 64
J = NH * D       # 512 (local q/k/v width)
JC = J // P      # 4 j-chunks


def build_program(T=2048):
    LC = T // P          # l/t 128-blocks (16)
    NS = T // 512        # 512-wide i-supers (4)
    SCALE = 0.125        # 1/sqrt(D)

    nc = bacc.Bacc("TRN2", target_bir_lowering=False, debug=False, num_devices=8)

    xT_d = nc.dram_tensor("xT", [CC, P, T], BF16, kind="ExternalInput")
    wqkvT_d = nc.dram_tensor("wqkvT", [CC, P, 3 * J], BF16, kind="ExternalInput")
    woutT_d = nc.dram_tensor("woutT", [JC, P, C], BF16, kind="ExternalInput")
    ident_d = nc.dram_tensor("ident", [P, P], BF16, kind="ExternalInput")
    maskm_d = nc.dram_tensor("maskm", [P, P], BF16, kind="ExternalInput")
    y_d = nc.dram_tensor("y", [LC, P, C], F32, kind="ExternalOutput")

    with tile.TileContext(nc) as tc:
        with (
            tc.tile_pool(name="persist", bufs=1) as persist,
            tc.tile_pool(name="io", bufs=1) as io_pool,
            tc.tile_pool(name="dst", bufs=3) as dst_pool,
            tc.tile_pool(name="dnp", bufs=2) as dn_pool,
            tc.tile_pool(name="rec", bufs=2) as rec_pool,
            tc.tile_pool(name="recb", bufs=2) as recb_pool,
            tc.tile_pool(name="expp", bufs=4) as exp_pool,
            tc.tile_pool(name="outp", bufs=3) as out_pool,
            tc.tile_pool(name="ps_m", bufs=4, space="PSUM") as ps_m,
            tc.tile_pool(name="ps_pv", bufs=2, space="PSUM") as ps_pv,
            tc.tile_pool(name="ps_bc", bufs=2, space="PSUM") as ps_bc,
        ):
            woutT = persist.tile([P, JC, C], BF16)
            ident = persist.tile([P, P], BF16)
            maskm = persist.tile([P, P], BF16)
            ones_t = persist.tile([1, D], BF16)
            qkT = persist.tile([P, 2 * JC, T], BF16)
            v_aug = persist.tile([P, LC, NH, D + 1], BF16)
            attnT = persist.tile([P, JC, T], BF16)
            xT = io_pool.tile([P, CC, T], BF16)
            wqkvT = io_pool.tile([P, CC, 3 * J], BF16)

            # inputs: tiny mask/ident first, x/w chunks interleaved, wout last
            nc.sync.dma_start(ident[:], ident_d[:])
            nc.sync.dma_start(maskm[:], maskm_d[:])
            for cc in range(CC):
                nc.sync.dma_start(xT[:, cc, :], xT_d[cc])
                nc.sync.dma_start(wqkvT[:, cc, :], wqkvT_d[cc])
            for jc in range(JC):
                nc.sync.dma_start(woutT[:, jc, :], woutT_d[jc])
            nc.gpsimd.memset(ones_t[:], 1.0)
            nc.gpsimd.memset(v_aug[:, :, :, D], 1.0)

            # ---------------- QKV projection pieces ----------------
            def emit_qk_tile(jc, ts):
                """q/k chunk jc (0..7: q then k), t-super ts: [P, 512]."""
                pq = ps_m.tile([P, 512], F32, tag="m", name=f"qk{jc}_{ts}")
                for cc in range(CC):
                    nc.tensor.matmul(
                        pq[:],
                        wqkvT[:, cc, jc * P : (jc + 1) * P],
                        xT[:, cc, ts * 512 : (ts + 1) * 512],
                        start=(cc == 0),
                        stop=(cc == CC - 1),
                    )
                nc.vector.tensor_copy(
                    qkT[:, jc, ts * 512 : (ts + 1) * 512], pq[:]
                )

            def emit_v_tile(lc):
                """v for t-block lc into v_aug rows 1..64."""
                pq = ps_m.tile([P, 512], F32, tag="m", name=f"v{lc}")
                for cc in range(CC):
                    nc.tensor.matmul(
                        pq[:],
                        xT[:, cc, lc * P : (lc + 1) * P],
                        wqkvT[:, cc, 2 * J : 3 * J],
                        start=(cc == 0),
                        stop=(cc == CC - 1),
                    )
                nc.vector.tensor_copy(
                    v_aug[:, lc, :, 0:D],
                    pq[:].rearrange("p (h d) -> p h d", d=D),
                )

            # load-phase backlog: pair-0 q/k (all supers) + v blocks 0..3
            for ts in range(NS):
                emit_qk_tile(0, ts)
                emit_qk_tile(JC, ts)
                if ts < 2:
                    emit_v_tile(2 * ts)
                    emit_v_tile(2 * ts + 1)

            # ---------------- attention, super-major ----------------
            def emit_head_super(h, S):
                bp = (h % 2) * 64
                chq = h // 2
                qTh = qkT[bp : bp + 64, chq, :]
                kTh = qkT[bp : bp + 64, JC + chq, :]
                cs, ce = S * 512, (S + 1) * 512
                pv = ps_pv.tile([D + 1, 512], F32, tag="pv", name=f"pv{h}_{S}")

                for lb in range(4 * S + 4):
                    l0 = lb * P
                    lo = max(l0, cs)
                    n = ce - lo
                    sc = ps_m.tile([P, 512], F32, tag="m", name=f"sc{h}_{S}_{lb}")
                    ex = exp_pool.tile([P, 512], BF16, tag="ex", name=f"ex{h}_{S}_{lb}")
                    if l0 >= cs:
                        # diagonal block: score matmul + additive causal mask
                        nc.tensor.matmul(
                            sc[:, 0:P], kTh[:, l0 : l0 + P], qTh[:, lo : lo + P],
                            start=True, stop=False,
                        )
                        nc.tensor.matmul(
                            sc[:, 0:P], ident[:], maskm[:], start=False, stop=True,
                        )
                        if n > P:
                            nc.tensor.matmul(
                                sc[:, P:n], kTh[:, l0 : l0 + P], qTh[:, lo + P : ce],
                                start=True, stop=True,
                            )
                    else:
                        nc.tensor.matmul(
                            sc[:, 0:n], kTh[:, l0 : l0 + P], qTh[:, lo:ce],
                            start=True, stop=True,
                        )
                    nc.scalar.activation(ex[:, 0:n], sc[:, 0:n], EXPF, scale=SCALE)
                    nc.tensor.matmul(
                        pv[:, 512 - n :],
                        v_aug[:, lb, h, :],
                        ex[:, 0:n],
                        start=(lb == 0),
                        stop=(lb == 4 * S + 3),
                    )

                # normalize this (head, super): row 64 of pv is the denominator
                dstage = dst_pool.tile([D, 512], F32, tag="dn", name=f"dn{h}_{S}")
                dn = dn_pool.tile([1, 512], F32, tag="dd", name=f"dd{h}_{S}")
                rec = rec_pool.tile([1, 512], F32, tag="rc", name=f"rc{h}_{S}")
                recb = recb_pool.tile([1, 512], BF16, tag="rb", name=f"rb{h}_{S}")
                bc = ps_bc.tile([D, 512], F32, tag="bc", name=f"bc{h}_{S}")
                nc.vector.tensor_copy(dn[:], pv[D : D + 1, :])
                nc.vector.reciprocal_approx_fast(rec[:], dn[:])
                nc.vector.tensor_copy(recb[:], rec[:])
                nc.vector.tensor_copy(dstage[:], pv[0:D, :])
                nc.tensor.matmul(
                    bc[:], ones_t[0:1, :], recb[0:1, :], start=True, stop=True
                )
                nc.vector.tensor_mul(
                    attnT[bp : bp + 64, chq, cs:ce], dstage[:], bc[:]
                )

            def emit_outproj_super(S):
                for tb in range(4 * S, 4 * S + 4):
                    for oc in range(2):
                        po = ps_m.tile([P, 512], F32, tag="m", name=f"o{tb}_{oc}")
                        for jc in range(JC):
                            nc.tensor.matmul(
                                po[:],
                                attnT[:, jc, tb * P : (tb + 1) * P],
                                woutT[:, jc, oc * 512 : (oc + 1) * 512],
                                start=(jc == 0),
                                stop=(jc == JC - 1),
                            )
                        ot = out_pool.tile([P, 512], F32, tag="ot", name=f"ot{tb}_{oc}")
                        nc.vector.tensor_copy(ot[:], po[:])
                        nc.sync.dma_start(
                            y_d[tb][:, oc * 512 : (oc + 1) * 512], ot[:]
                        )

            for S in range(NS):
                for p in range(JC):
                    if S == 0 and p > 0:
                        # this pair's super-0 q/k tiles (needed right now)
                        emit_qk_tile(p, 0)
                        emit_qk_tile(JC + p, 0)
                    if S + 1 < NS:
                        # next super's q/k tiles for this head pair (filler)
                        emit_qk_tile(p, S + 1)
                        emit_qk_tile(JC + p, S + 1)
                    emit_head_super(2 * p, S)
                    emit_head_super(2 * p + 1, S)
                if S > 0:
                    emit_outproj_super(S - 1)
                if S + 1 < NS:
                    # v blocks needed from super S+1 (lb up to 4(S+1)+3)
                    for lc in range(4 * S + 4, 4 * S + 8):
                        emit_v_tile(lc)
            emit_outproj_super(NS - 1)

    nc.compile()
    return nc


_CACHE = {}

# Set by test harnesses to capture a profile; harmless defaults for grading.
TRACE = False
LAST_RESULT = None


def get_program(T=2048):
    if T not in _CACHE:
        _CACHE[T] = build_program(T)
    return _CACHE[T]


def make_in_map(x_b, w_qkv, w_out, hg, T=2048):
    """Host-side shard prep for one core: batch slice x_b [T, C], head group hg."""
    xT = np.ascontiguousarray(x_b.T).astype(NPBF16).reshape(CC, P, T)
    W = np.concatenate(
        [
            w_qkv[hg * J : (hg + 1) * J],
            w_qkv[C + hg * J : C + (hg + 1) * J],
            w_qkv[2 * C + hg * J : 2 * C + (hg + 1) * J],
        ],
        axis=0,
    )  # [3J, C]
    wqkvT = np.ascontiguousarray(W.T).astype(NPBF16).reshape(CC, P, 3 * J)
    Wo = w_out[:, hg * J : (hg + 1) * J]  # [C, J]
    woutT = np.ascontiguousarray(Wo.T).astype(NPBF16).reshape(JC, P, C)
    ident = np.eye(P, dtype=np.float32).astype(NPBF16)
    # additive causal mask for the diagonal 128-block, [l_local, i_local]:
    # invalid where i_local < l_local
    maskm = np.where(
        np.arange(P)[None, :] < np.arange(P)[:, None], -1e6, 0.0
    ).astype(NPBF16)
    return {"xT": xT, "wqkvT": wqkvT, "woutT": woutT, "ident": ident, "maskm": maskm}


def kernel(x, w_qkv, w_out, b_out):
    x = np.asarray(x, dtype=np.float32)
    w_qkv = np.asarray(w_qkv, dtype=np.float32)
    w_out = np.asarray(w_out, dtype=np.float32)
    b_out = np.asarray(b_out, dtype=np.float32)
    B, T, Cx = x.shape
    assert Cx == C

    nc = get_program(T)
    in_maps = [
        make_in_map(x[core // 2], w_qkv, w_out, core % 2, T) for core in range(8)
    ]
    res = run_bass_kernel_spmd(nc, in_maps, core_ids=list(range(8)), trace=TRACE)
    global LAST_RESULT
    LAST_RESULT = res
    outs = [r["y"].reshape(T, C).astype(np.float32) for r in res.results]
    y = np.stack([outs[2 * b] + outs[2 * b + 1] for b in range(B)])
    return (y + b_out[None, None, :]).astype(np.float32)
